# revision 1
# baseline (speedup 1.0000x reference)
"""Fused QKV-projection + multi-head attention kernel for Trainium2.

Problem: x [2, 2048, 1024] fp32; W_qkv [1024, 3072]; b_qkv [3072].
  qkv = x @ W_qkv + b; 16 heads of 64; scores = q k^T / 8; softmax; out = attn @ v.

Sharding: 8 cores = 2 (batch) x 4 (head groups of 4). Each core is fully
independent (no collectives): projection for its batch restricted to its 4
heads' q/k/v columns, then attention for those heads.

Per-core design:
  - host feeds x^T with an appended ones-row; biases fold into the weights as
    an extra contraction row, so projection = one matmul chain, bias included.
  - q/k are produced TRANSPOSED and packed in head-PAIR tiles [128, S]
    (rows 0-63 = even head, 64-127 = odd head). The two halves drive two
    matmuls on disjoint PE row-groups (tile_position auto-derived from the
    base partition) that execute CONCURRENTLY on the 128x128 array --
    recovering the half-array loss of the Dh=64 contraction.
  - scores^T = kT.T @ qT needs no transposes anywhere; softmax is a single
    fused exp on ScalarE (scale=1/8 applied by the ACT datapath; no max
    subtraction -- scores are O(+-8), well within fp16/fp32 exp range).
  - V is produced in natural layout with an extra constant-1 column generated
    by the same augmented projection: accumulating O^T_aug = V_aug.T @ exp(S^T)
    yields the attention output AND the softmax denominators in one stream.
  - normalization is transpose-free: the denominator row is broadcast across
    partitions via a DRAM bounce, then reciprocal+multiply on VectorE; the
    output is stored TRANSPOSED [head-dim, seq] and flipped on the host.
  - matmul operands are fp16 (measured end-to-end rel err ~1.1e-3 vs the fp32
    reference; strict-fp32 matmuls are 4x slower, fp32r trips walrus sync
    limits). PSUM accumulation is fp32; softmax denominators (~5e6) would
    overflow fp16 so they stay fp32 throughout.

Scheduling: Tile tracks dependencies in EMISSION order, while scheduler
priority is tc.cur_priority -- V production is emitted early (correct deps)
but in a low-priority band so it fills PE slack behind the ACT-bound exp
stream; the pair-1 projection overlaps pair-0's attention the same way.

Sync: this walrus build rejects instructions carrying more than one embedded
semaphore wait. _relax_waits() strips provably redundant waits (PE self-waits;
same-engine-covered waits) and _split_multi_waits() hoists any remaining
excess onto single-wait NoOps inserted before the instruction.
"""

import sys

if "/opt/trn_rl_repo" not in sys.path:
    sys.path.insert(0, "/opt/trn_rl_repo")

import numpy as np
from contextlib import ExitStack

B, S, D = 2, 2048, 1024
H, Dh = 16, 64
HL = 4          # heads per core
GW = HL * Dh    # 256 output cols per core
VW = HL * 65    # V_aug width: per head [v (64) | ones (1)]
KC = 1025       # augmented contraction (1024 + bias row)
NST = S // 128  # 16 s-tiles
NQB = S // 512  # 4 q blocks

_CACHE = {}


def _build_nc():
    import concourse.bass as bass
    import concourse.mybir as mybir
    import concourse.tile as tile

    f32 = mybir.dt.float32
    f16 = mybir.dt.float16
    Exp = mybir.ActivationFunctionType.Exp

    nc = bass.Bass()
    xT = nc.dram_tensor("xT", [KC, S], f16, kind="ExternalInput")
    wqk = nc.dram_tensor("wqk", [KC, 512], f16, kind="ExternalInput")
    wv = nc.dram_tensor("wv", [KC, VW], f16, kind="ExternalInput")
    # output is stored TRANSPOSED [head-dim, seq]; host transposes back
    out = nc.dram_tensor("out", [GW, S], f32, kind="ExternalOutput")

    def chunks():
        for d in range(9):
            yield d, (128 if d < 8 else 1)

    with tile.TileContext(nc) as tc, ExitStack() as ctx:
        persist = ctx.enter_context(tc.tile_pool(name="persist", bufs=1))
        # q/k head-PAIR tiles: rows 0..63 = head 2p, 64..127 = head 2p+1.
        # The two 64-row halves drive two concurrent matmuls on disjoint
        # PE row-groups (tile_position auto-derived from base partition).
        qP = [persist.tile([128, S], f16, name=f"qP{p}", tag=f"qP{p}") for p in range(2)]
        kP = [persist.tile([128, S], f16, name=f"kP{p}", tag=f"kP{p}") for p in range(2)]
        V = [persist.tile([128, VW], f16, name=f"V{t}", tag=f"V{t}") for t in range(NST)]
        ones64 = persist.tile([1, 64], f16, name="ones64", tag="ones64")
        nc.vector.memset(ones64, 1.0)

        wpool = ctx.enter_context(tc.tile_pool(name="wpool", bufs=1))
        xpool = ctx.enter_context(tc.tile_pool(name="xpool", bufs=1))

        # all input DMAs up front, interleaved so early chunks land first
        wqk_sb, wv_sb, xs = [], [], [[], []]
        for d, p in chunks():
            twq = wpool.tile([p, 512], f16, name=f"wq{d}", tag=f"wq{d}")
            nc.sync.dma_start(out=twq, in_=wqk[d * 128:d * 128 + p, :])
            wqk_sb.append(twq)
            t0 = xpool.tile([p, 1024], f16, name=f"x0_{d}", tag=f"x0_{d}")
            nc.sync.dma_start(out=t0, in_=xT[d * 128:d * 128 + p, 0:1024])
            xs[0].append(t0)
        for d, p in chunks():
            t1 = xpool.tile([p, 1024], f16, name=f"x1_{d}", tag=f"x1_{d}")
            nc.sync.dma_start(out=t1, in_=xT[d * 128:d * 128 + p, 1024:2048])
            xs[1].append(t1)
        for d, p in chunks():
            twv = wpool.tile([p, VW], f16, name=f"wv{d}", tag=f"wv{d}")
            nc.sync.dma_start(out=twv, in_=wv[d * 128:d * 128 + p, :])
            wv_sb.append(twv)

        with tc.tile_pool(name="psA", bufs=2, space="PSUM") as psA, \
             tc.tile_pool(name="expp", bufs=48) as expp, \
             tc.tile_pool(name="normp", bufs=3) as normp, \
             tc.tile_pool(name="dscr", bufs=3, space="DRAM") as dscr_pool, \
             tc.tile_pool(name="psS", bufs=2, space="PSUM") as psS, \
             tc.tile_pool(name="psO", bufs=2, space="PSUM") as psO:

            def qk_half(sh, mt, j):
                """Half a projection M-tile (one 512-col q-block) -> qP/kP.
                mt 0/1 -> q pairs 0/1, mt 2/3 -> k pairs 0/1."""
                dst = (qP if mt < 2 else kP)[mt % 2]
                ps = psA.tile([128, 512], f32, name=f"psA{sh}_{mt}_{j}", tag="psA")
                for d, p in chunks():
                    nc.tensor.matmul(ps, wqk_sb[d][:, mt * 128:(mt + 1) * 128],
                                     xs[sh][d][:, j * 512:(j + 1) * 512],
                                     start=(d == 0), stop=(d == 8))
                qb = sh * 2 + j
                nc.vector.tensor_copy(dst[:, qb * 512:(qb + 1) * 512], ps)

            def qk_group(sh, mt):
                qk_half(sh, mt, 0)
                qk_half(sh, mt, 1)

            def v_group(st):
                sh, stl = divmod(st, 8)
                psv = psA.tile([128, VW], f32, name=f"psV{st}", tag="psA")
                for d, p in chunks():
                    nc.tensor.matmul(psv,
                                     xs[sh][d][:, stl * 128:(stl + 1) * 128],
                                     wv_sb[d], start=(d == 0), stop=(d == 8))
                nc.vector.tensor_copy(V[st], psv)

            def attention_iter(p, qb, last=False):
                # packed scores^T: head 2p on PE rows 0-63 -> psS bank 0,
                # head 2p+1 on rows 64-127 -> bank 1; one fused exp over both
                ets = []
                for st in range(NST):
                    ps = psS.tile([128, 1024], f32, name=f"s{p}_{qb}_{st}", tag="psS")
                    for hh in range(2):
                        r0, r1 = hh * 64, hh * 64 + 64
                        nc.tensor.matmul(
                            ps[:, hh * 512:(hh + 1) * 512],
                            kP[p][r0:r1, st * 128:(st + 1) * 128],
                            qP[p][r0:r1, qb * 512:(qb + 1) * 512],
                            start=True, stop=True)
                    et = expp.tile([128, 1024], f16, name=f"e{p}_{qb}_{st}", tag="expS")
                    nc.scalar.activation(et, ps, Exp, scale=0.125)
                    ets.append(et)

                # O^T_aug accumulation per head: rows 0..63 out, row 64 denom
                po = [psO.tile([65, 512], f32, name=f"po{p}_{qb}_{hh}", tag="psO")
                      for hh in range(2)]
                for st in range(NST):
                    for hh in range(2):
                        nc.tensor.matmul(
                            po[hh], V[st][:, (2 * p + hh) * 65:(2 * p + hh + 1) * 65],
                            ets[st][:, hh * 512:(hh + 1) * 512],
                            start=(st == 0), stop=(st == NST - 1))

                # transpose-free normalize: broadcast the denominator row
                # across partitions via DMA, reciprocal + multiply on DVE,
                # store transposed (host flips back)
                for hh in range(2):
                    h = 2 * p + hh
                    drow = normp.tile([1, 512], f32, name=f"dr{p}_{qb}_{hh}", tag="drow")
                    nc.vector.reciprocal(drow, po[hh][64:65, :])
                    if last:
                        # tail-latency path: broadcast via a K=1 ones-matmul
                        # into a (now idle) psS slot -- no DRAM round trip.
                        # (DVE may read only ONE psum operand, so the numerator
                        # rows bounce to SBUF; that copy overlaps the chain.)
                        drow16 = normp.tile([1, 512], f16, name=f"dh{p}_{qb}_{hh}",
                                            tag="drow16")
                        nc.vector.tensor_copy(drow16, drow)
                        recp = psS.tile([64, 512], f32, name=f"rp{p}_{qb}_{hh}",
                                        tag="psS")
                        nc.tensor.matmul(recp, ones64, drow16,
                                         start=True, stop=True)
                        osb = normp.tile([64, 512], f32, name=f"ob{p}_{qb}_{hh}",
                                         tag="osb")
                        nc.vector.tensor_copy(osb, po[hh][0:64, :])
                        po_src, rec_ap = osb, recp
                    else:
                        # steady-state path: partition-broadcast via a DRAM
                        # bounce (engines/DMA cannot zero-step SBUF partitions,
                        # but a DRAM source can be read by all 64 partitions)
                        dscr = dscr_pool.tile([1, 512], f32, name=f"ds{p}_{qb}_{hh}",
                                              tag="dscr")
                        nc.sync.dma_start(out=dscr, in_=drow)
                        bcast = bass.AP(tensor=dscr.tensor, offset=dscr.offset,
                                        ap=[[0, 64]] + list(dscr.ap)[1:])
                        rec_ap = normp.tile([64, 512], f32, name=f"rc{p}_{qb}_{hh}",
                                            tag="rec")
                        nc.sync.dma_start(out=rec_ap, in_=bcast)
                        po_src = po[hh][0:64, :]
                    otn = normp.tile([64, 512], f32, name=f"on{p}_{qb}_{hh}", tag="otn")
                    nc.vector.tensor_mul(otn, po_src, rec_ap)
                    nc.sync.dma_start(
                        out=out[h * 64:(h + 1) * 64, qb * 512:(qb + 1) * 512],
                        in_=otn)

            # Dependency tracking is emission-order based: every producer must
            # be emitted before its consumers. Scheduling PRIORITY, however, is
            # tc.cur_priority, which we can band-shift: V is emitted early (so
            # PV sees its writes) but in a low-priority band, making it PE
            # slack-filler behind the ACT-feeding scores stream.
            qk_group(0, 0)
            qk_group(0, 2)
            qk_group(1, 0)
            qk_group(1, 2)
            p_save = tc.cur_priority
            tc.cur_priority = p_save + 600
            for st in range(NST):
                v_group(st)
            tc.cur_priority += 600
            qk_group(0, 1)
            qk_group(0, 3)
            qk_group(1, 1)
            qk_group(1, 3)
            p_proj_end = tc.cur_priority
            tc.cur_priority = p_save
            for qb in range(NQB):
                attention_iter(0, qb)
            tc.cur_priority = max(tc.cur_priority, p_proj_end)
            for qb in range(NQB):
                attention_iter(1, qb, last=(qb == NQB - 1))
    return nc


def _relax_waits(nc):
    """Walrus rejects instructions carrying more than ~1 embedded semaphore
    wait ("Too many sync wait commands"). Strip waits that are provably
    redundant. Soundness (this kernel is fully unrolled: no loops, no sem
    resets, all sems monotone):
      R1: a PE instruction never needs a wait on PE's own completion
          semaphore: PE executes in order, never reads its own output
          (no PSUM read port), and drains (PSUM writes) are in order.
      R2: a wait (sem >= v) is redundant if an earlier instruction on the
          same engine already waits (sem >= v' >= v): the per-engine
          sequencer processes waits in stream order.
    Returns the number of instructions still carrying >1 ge-waits."""
    # Only PE: it never reads its own writes (no PSUM read port), and its
    # in-order drain sequences PSUM WAW. DVE/ACT have deep non-interlocked
    # pipelines -- their self-waits guard real RAW hazards.
    own_sem = {"PE": "PE_"}
    observed = {}  # (engine, sem id) -> max value waited
    remaining = 0
    for fn in nc.m.functions:
        for blk in fn.blocks:
            for inst in blk.instructions:
                si = getattr(inst, "sync_info", None)
                if si is None or not si.on_wait:
                    continue
                eng = str(inst.engine).split(".")[-1]
                pfx = own_sem.get(eng)
                keep, nge = [], 0
                for w in si.on_wait:
                    if w.sync_type != "semaphore" or w.wait_mode != "sem-ge-imm" \
                            or w.wait_reg is not None \
                            or w.ant_name.startswith("barrier_"):
                        # barrier sems are decremented (non-monotone): hands off
                        keep.append(w)
                        continue
                    if pfx is not None and w.ant_name.startswith(pfx):
                        continue  # R1
                    k = (eng, w.id)
                    if observed.get(k, -1) >= w.wait_value:
                        continue  # R2
                    observed[k] = w.wait_value
                    keep.append(w)
                    nge += 1
                if nge > 1:
                    remaining += 1
                if len(keep) != len(si.on_wait):
                    si.on_wait = keep
                    inst.sync_info = si
    return remaining


def _split_multi_waits(nc):
    """Any instruction still carrying >1 ge-waits after relaxation gets its
    excess waits hoisted onto same-engine NoOps inserted right before it
    (a sequence of single-wait instructions is semantically identical to one
    multi-wait instruction on an in-order sequencer)."""
    import bass_rust

    def wkey(w):
        return (w.id, w.wait_value, w.wait_mode)

    plan = {}
    for fn in nc.m.functions:
        for blk in fn.blocks:
            for inst in blk.instructions:
                si = getattr(inst, "sync_info", None)
                if si is None or not si.on_wait:
                    continue
                ow = list(si.on_wait)
                ge = [w for w in ow
                      if w.sync_type == "semaphore" and w.wait_mode == "sem-ge-imm"
                      and w.wait_reg is None
                      and not w.ant_name.startswith("barrier_")]
                if len(ge) <= 1:
                    continue
                hoist = ge[1:]
                hkeys = {wkey(w) for w in hoist}
                nops = []
                for w in hoist:
                    nb = nc.engines[inst.engine].nop(nofuse=True, hint="wait_split")
                    ni = nb.ins
                    ni.sync_info = bass_rust.SyncInfo(on_wait=[w], on_update=[])
                    nops.append(ni)
                plan[inst.name] = nops
                si.on_wait = [w for w in ow if wkey(w) not in hkeys
                              or (w.sync_type, w.wait_mode) != ("semaphore", "sem-ge-imm")]
                inst.sync_info = si
    if not plan:
        return 0
    created = {n.name for nops in plan.values() for n in nops}
    for fn in nc.m.functions:
        for blk in fn.blocks:
            cur = list(blk.instructions)
            new = []
            for i in cur:
                if i.name in created:
                    continue
                if i.name in plan:
                    new.extend(plan[i.name])
                new.append(i)
            blk.instructions = new
    return len(plan)


def get_nc():
    if "nc" not in _CACHE:
        nc = _build_nc()
        _relax_waits(nc)
        _split_multi_waits(nc)
        _CACHE["nc"] = nc
    return _CACHE["nc"]


def prep_inputs(x, W_qkv, b_qkv):
    """Host-side sharding: returns the 8 per-core input maps."""
    x = np.asarray(x, dtype=np.float32)
    W_qkv = np.asarray(W_qkv, dtype=np.float32)
    b_qkv = np.asarray(b_qkv, dtype=np.float32)
    ones = np.ones((1, S), np.float32)
    in_maps = []
    for c in range(8):
        b, g = divmod(c, 4)
        xTm = np.concatenate([np.ascontiguousarray(x[b].T), ones], axis=0).astype(np.float16)
        heads = list(range(HL * g, HL * g + HL))
        cols = np.concatenate([np.arange(h * Dh, (h + 1) * Dh) for h in heads])
        wqk_m = np.empty((KC, 512), np.float16)
        wqk_m[:D, :256] = W_qkv[:, cols]
        wqk_m[D, :256] = b_qkv[cols]
        wqk_m[:D, 256:] = W_qkv[:, D + cols]
        wqk_m[D, 256:] = b_qkv[D + cols]
        wv_m = np.zeros((KC, VW), np.float16)
        for i, h in enumerate(heads):
            vcols = 2 * D + h * Dh
            wv_m[:D, i * 65:i * 65 + 64] = W_qkv[:, vcols:vcols + Dh]
            wv_m[D, i * 65:i * 65 + 64] = b_qkv[vcols:vcols + Dh]
            wv_m[D, i * 65 + 64] = 1.0  # generates the constant-1 denom column
        in_maps.append({"xT": xTm, "wqk": wqk_m, "wv": wv_m})
    return in_maps


def assemble_output(results):
    out = np.empty((B, S, D), np.float32)
    for c in range(8):
        b, g = divmod(c, 4)
        out[b, :, g * GW:(g + 1) * GW] = results[c]["out"].T
    return out


def kernel(x, W_qkv, b_qkv):
    from concourse.bass_utils import run_bass_kernel_spmd

    nc = get_nc()
    in_maps = prep_inputs(x, W_qkv, b_qkv)
    res = run_bass_kernel_spmd(nc, in_maps, list(range(8)))
    return assemble_output(res.results)



# revision 2
# speedup vs baseline: 253.2646x; 253.2646x over previous
"""Fused QKV-projection + multi-head attention kernel for Trainium2.

Problem: x [2, 2048, 1024] fp32; W_qkv [1024, 3072]; b_qkv [3072].
  qkv = x @ W_qkv + b; 16 heads of 64; scores = q k^T / 8; softmax; out = attn @ v.

Sharding: 8 cores = 2 (batch) x 4 (head groups of 4). Each core is fully
independent (no collectives): projection for its batch restricted to its 4
heads' q/k/v columns, then attention for those heads.

Per-core design:
  - host feeds x^T with an appended ones-row; biases fold into the weights as
    an extra contraction row, so projection = one matmul chain, bias included.
  - q/k are produced TRANSPOSED and packed in head-PAIR tiles [128, S]
    (rows 0-63 = even head, 64-127 = odd head). The two halves drive two
    matmuls on disjoint PE row-groups (tile_position auto-derived from the
    base partition) that execute CONCURRENTLY on the 128x128 array --
    recovering the half-array loss of the Dh=64 contraction.
  - scores^T = kT.T @ qT needs no transposes anywhere; softmax is a single
    fused exp on ScalarE (scale=1/8 applied by the ACT datapath; no max
    subtraction -- scores are O(+-8), well within fp16/fp32 exp range).
  - V is produced in natural layout with an extra constant-1 column generated
    by the same augmented projection: accumulating O^T_aug = V_aug.T @ exp(S^T)
    yields the attention output AND the softmax denominators in one stream.
  - normalization is transpose-free: the denominator row is broadcast across
    partitions via a DRAM bounce, then reciprocal+multiply on VectorE; the
    output is stored TRANSPOSED [head-dim, seq] and flipped on the host.
  - matmul operands are fp16 (measured end-to-end rel err ~1.1e-3 vs the fp32
    reference; strict-fp32 matmuls are 4x slower, fp32r trips walrus sync
    limits). PSUM accumulation is fp32; softmax denominators (~5e6) would
    overflow fp16 so they stay fp32 throughout.

Scheduling: Tile tracks dependencies in EMISSION order, while scheduler
priority is tc.cur_priority -- V production is emitted early (correct deps)
but in a low-priority band so it fills PE slack behind the ACT-bound exp
stream; the pair-1 projection overlaps pair-0's attention the same way.

Sync: this walrus build rejects instructions carrying more than one embedded
semaphore wait. _relax_waits() strips provably redundant waits (PE self-waits;
same-engine-covered waits) and _split_multi_waits() hoists any remaining
excess onto single-wait NoOps inserted before the instruction.
"""

import sys

if "/opt/trn_rl_repo" not in sys.path:
    sys.path.insert(0, "/opt/trn_rl_repo")

import numpy as np
from contextlib import ExitStack

B, S, D = 2, 2048, 1024
H, Dh = 16, 64
HL = 4          # heads per core
GW = HL * Dh    # 256 output cols per core
VW = HL * 65    # V_aug width: per head [v (64) | ones (1)]
KC = 1025       # augmented contraction (1024 + bias row)
NST = S // 128  # 16 s-tiles
NQB = S // 512  # 4 q blocks

_CACHE = {}


def _build_nc(repeat=1):
    """Build the kernel module. repeat>1 unrolls the WHOLE computation that
    many times back-to-back (input DMAs included; SBUF/PSUM pool buffers are
    reused across iterations via tag rotation) so that steady-state
    per-iteration device time can be measured through the high-latency axon
    tunnel. kernel() always uses repeat=1."""
    import concourse.bass as bass
    import concourse.mybir as mybir
    import concourse.tile as tile

    f32 = mybir.dt.float32
    f16 = mybir.dt.float16
    Exp = mybir.ActivationFunctionType.Exp

    nc = bass.Bass()
    xT = nc.dram_tensor("xT", [KC, S], f16, kind="ExternalInput")
    wqk = nc.dram_tensor("wqk", [KC, 512], f16, kind="ExternalInput")
    wv = nc.dram_tensor("wv", [KC, VW], f16, kind="ExternalInput")
    # output is stored TRANSPOSED [head-dim, seq]; host transposes back
    out = nc.dram_tensor("out", [GW, S], f32, kind="ExternalOutput")

    def chunks():
        for d in range(9):
            yield d, (128 if d < 8 else 1)

    with tile.TileContext(nc) as tc, ExitStack() as ctx:
        persist = ctx.enter_context(tc.tile_pool(name="persist", bufs=1))
        ones64 = persist.tile([1, 64], f16, name="ones64", tag="ones64")
        nc.vector.memset(ones64, 1.0)

        wpool = ctx.enter_context(tc.tile_pool(name="wpool", bufs=1))
        xpool = ctx.enter_context(tc.tile_pool(name="xpool", bufs=1))
        psA = ctx.enter_context(tc.tile_pool(name="psA", bufs=2, space="PSUM"))
        expp = ctx.enter_context(tc.tile_pool(name="expp", bufs=48))
        normp = ctx.enter_context(tc.tile_pool(name="normp", bufs=3))
        dscr_pool = ctx.enter_context(tc.tile_pool(name="dscr", bufs=3, space="DRAM"))
        psS = ctx.enter_context(tc.tile_pool(name="psS", bufs=2, space="PSUM"))
        psO = ctx.enter_context(tc.tile_pool(name="psO", bufs=2, space="PSUM"))

        for rep in range(repeat):
            rr = f"r{rep}_" if repeat > 1 else ""

            # q/k head-PAIR tiles: rows 0..63 = head 2p, 64..127 = head 2p+1.
            # The two 64-row halves drive two concurrent matmuls on disjoint
            # PE row-groups (tile_position auto-derived from base partition).
            qP = [persist.tile([128, S], f16, name=f"{rr}qP{p}", tag=f"qP{p}")
                  for p in range(2)]
            kP = [persist.tile([128, S], f16, name=f"{rr}kP{p}", tag=f"kP{p}")
                  for p in range(2)]
            V = [persist.tile([128, VW], f16, name=f"{rr}V{t}", tag=f"V{t}")
                 for t in range(NST)]

            # all input DMAs up front, interleaved so early chunks land first
            wqk_sb, wv_sb, xs = [], [], [[], []]
            for d, p in chunks():
                twq = wpool.tile([p, 512], f16, name=f"{rr}wq{d}", tag=f"wq{d}")
                nc.sync.dma_start(out=twq, in_=wqk[d * 128:d * 128 + p, :])
                wqk_sb.append(twq)
                t0 = xpool.tile([p, 1024], f16, name=f"{rr}x0_{d}", tag=f"x0_{d}")
                nc.sync.dma_start(out=t0, in_=xT[d * 128:d * 128 + p, 0:1024])
                xs[0].append(t0)
            for d, p in chunks():
                t1 = xpool.tile([p, 1024], f16, name=f"{rr}x1_{d}", tag=f"x1_{d}")
                nc.sync.dma_start(out=t1, in_=xT[d * 128:d * 128 + p, 1024:2048])
                xs[1].append(t1)
            for d, p in chunks():
                twv = wpool.tile([p, VW], f16, name=f"{rr}wv{d}", tag=f"wv{d}")
                nc.sync.dma_start(out=twv, in_=wv[d * 128:d * 128 + p, :])
                wv_sb.append(twv)

            def qk_half(sh, mt, j):
                """Half a projection M-tile (one 512-col q-block) -> qP/kP.
                mt 0/1 -> q pairs 0/1, mt 2/3 -> k pairs 0/1."""
                dst = (qP if mt < 2 else kP)[mt % 2]
                ps = psA.tile([128, 512], f32, name=f"{rr}psA{sh}_{mt}_{j}", tag="psA")
                for d, p in chunks():
                    nc.tensor.matmul(ps, wqk_sb[d][:, mt * 128:(mt + 1) * 128],
                                     xs[sh][d][:, j * 512:(j + 1) * 512],
                                     start=(d == 0), stop=(d == 8))
                qb = sh * 2 + j
                nc.vector.tensor_copy(dst[:, qb * 512:(qb + 1) * 512], ps)

            def qk_group(sh, mt):
                qk_half(sh, mt, 0)
                qk_half(sh, mt, 1)

            def v_group(st):
                sh, stl = divmod(st, 8)
                psv = psA.tile([128, VW], f32, name=f"{rr}psV{st}", tag="psA")
                for d, p in chunks():
                    nc.tensor.matmul(psv,
                                     xs[sh][d][:, stl * 128:(stl + 1) * 128],
                                     wv_sb[d], start=(d == 0), stop=(d == 8))
                nc.vector.tensor_copy(V[st], psv)

            def attention_iter(p, qb, last=False):
                # packed scores^T: head 2p on PE rows 0-63 -> psS bank 0,
                # head 2p+1 on rows 64-127 -> bank 1; one fused exp over both
                ets = []
                for st in range(NST):
                    ps = psS.tile([128, 1024], f32, name=f"{rr}s{p}_{qb}_{st}",
                                  tag="psS")
                    for hh in range(2):
                        r0, r1 = hh * 64, hh * 64 + 64
                        nc.tensor.matmul(
                            ps[:, hh * 512:(hh + 1) * 512],
                            kP[p][r0:r1, st * 128:(st + 1) * 128],
                            qP[p][r0:r1, qb * 512:(qb + 1) * 512],
                            start=True, stop=True)
                    et = expp.tile([128, 1024], f16, name=f"{rr}e{p}_{qb}_{st}",
                                   tag="expS")
                    nc.scalar.activation(et, ps, Exp, scale=0.125)
                    ets.append(et)

                # O^T_aug accumulation per head: rows 0..63 out, row 64 denom
                po = [psO.tile([65, 512], f32, name=f"{rr}po{p}_{qb}_{hh}", tag="psO")
                      for hh in range(2)]
                for st in range(NST):
                    for hh in range(2):
                        nc.tensor.matmul(
                            po[hh], V[st][:, (2 * p + hh) * 65:(2 * p + hh + 1) * 65],
                            ets[st][:, hh * 512:(hh + 1) * 512],
                            start=(st == 0), stop=(st == NST - 1))

                # transpose-free normalize: broadcast the denominator row
                # across partitions via DMA, reciprocal + multiply on DVE,
                # store transposed (host flips back)
                for hh in range(2):
                    h = 2 * p + hh
                    drow = normp.tile([1, 512], f32, name=f"{rr}dr{p}_{qb}_{hh}",
                                      tag="drow")
                    nc.vector.reciprocal(drow, po[hh][64:65, :])
                    if last:
                        # tail-latency path: broadcast via a K=1 ones-matmul
                        # into a (now idle) psS slot -- no DRAM round trip.
                        # (DVE may read only ONE psum operand, so the numerator
                        # rows bounce to SBUF; that copy overlaps the chain.)
                        drow16 = normp.tile([1, 512], f16,
                                            name=f"{rr}dh{p}_{qb}_{hh}", tag="drow16")
                        nc.vector.tensor_copy(drow16, drow)
                        recp = psS.tile([64, 512], f32, name=f"{rr}rp{p}_{qb}_{hh}",
                                        tag="psS")
                        nc.tensor.matmul(recp, ones64, drow16,
                                         start=True, stop=True)
                        osb = normp.tile([64, 512], f32, name=f"{rr}ob{p}_{qb}_{hh}",
                                         tag="osb")
                        nc.vector.tensor_copy(osb, po[hh][0:64, :])
                        po_src, rec_ap = osb, recp
                    else:
                        # steady-state path: partition-broadcast via a DRAM
                        # bounce (engines/DMA cannot zero-step SBUF partitions,
                        # but a DRAM source can be read by all 64 partitions)
                        dscr = dscr_pool.tile([1, 512], f32,
                                              name=f"{rr}ds{p}_{qb}_{hh}", tag="dscr")
                        nc.sync.dma_start(out=dscr, in_=drow)
                        bcast = bass.AP(tensor=dscr.tensor, offset=dscr.offset,
                                        ap=[[0, 64]] + list(dscr.ap)[1:])
                        rec_ap = normp.tile([64, 512], f32,
                                            name=f"{rr}rc{p}_{qb}_{hh}", tag="rec")
                        nc.sync.dma_start(out=rec_ap, in_=bcast)
                        po_src = po[hh][0:64, :]
                    otn = normp.tile([64, 512], f32, name=f"{rr}on{p}_{qb}_{hh}",
                                     tag="otn")
                    nc.vector.tensor_mul(otn, po_src, rec_ap)
                    nc.sync.dma_start(
                        out=out[h * 64:(h + 1) * 64, qb * 512:(qb + 1) * 512],
                        in_=otn)

            # Dependency tracking is emission-order based: every producer must
            # be emitted before its consumers. Scheduling PRIORITY, however, is
            # tc.cur_priority, which we can band-shift: V is emitted early (so
            # PV sees its writes) but in a low-priority band, making it PE
            # slack-filler behind the ACT-bound exp stream.
            qk_group(0, 0)
            qk_group(0, 2)
            qk_group(1, 0)
            qk_group(1, 2)
            p_save = tc.cur_priority
            tc.cur_priority = p_save + 600
            for st in range(NST):
                v_group(st)
            tc.cur_priority += 600
            qk_group(0, 1)
            qk_group(0, 3)
            qk_group(1, 1)
            qk_group(1, 3)
            p_proj_end = tc.cur_priority
            tc.cur_priority = p_save
            for qb in range(NQB):
                attention_iter(0, qb)
            tc.cur_priority = max(tc.cur_priority, p_proj_end)
            for qb in range(NQB):
                attention_iter(1, qb, last=(qb == NQB - 1))
            # next iteration's bands start above everything emitted so far
            tc.cur_priority = p_proj_end + 600
    return nc


def _relax_waits(nc):
    """Walrus rejects instructions carrying more than ~1 embedded semaphore
    wait ("Too many sync wait commands"). Strip waits that are provably
    redundant. Soundness (this kernel is fully unrolled: no loops, no sem
    resets, all sems monotone):
      R1: a PE instruction never needs a wait on PE's own completion
          semaphore: PE executes in order, never reads its own output
          (no PSUM read port), and drains (PSUM writes) are in order.
      R2: a wait (sem >= v) is redundant if an earlier instruction on the
          same engine already waits (sem >= v' >= v): the per-engine
          sequencer processes waits in stream order.
    Returns the number of instructions still carrying >1 ge-waits."""
    # Only PE: it never reads its own writes (no PSUM read port), and its
    # in-order drain sequences PSUM WAW. DVE/ACT have deep non-interlocked
    # pipelines -- their self-waits guard real RAW hazards.
    own_sem = {"PE": "PE_"}
    observed = {}  # (engine, sem id) -> max value waited
    remaining = 0
    for fn in nc.m.functions:
        for blk in fn.blocks:
            for inst in blk.instructions:
                si = getattr(inst, "sync_info", None)
                if si is None or not si.on_wait:
                    continue
                eng = str(inst.engine).split(".")[-1]
                pfx = own_sem.get(eng)
                keep, nge = [], 0
                for w in si.on_wait:
                    if w.sync_type != "semaphore" or w.wait_mode != "sem-ge-imm" \
                            or w.wait_reg is not None \
                            or w.ant_name.startswith("barrier_"):
                        # barrier sems are decremented (non-monotone): hands off
                        keep.append(w)
                        continue
                    if pfx is not None and w.ant_name.startswith(pfx):
                        continue  # R1
                    k = (eng, w.id)
                    if observed.get(k, -1) >= w.wait_value:
                        continue  # R2
                    observed[k] = w.wait_value
                    keep.append(w)
                    nge += 1
                if nge > 1:
                    remaining += 1
                if len(keep) != len(si.on_wait):
                    si.on_wait = keep
                    inst.sync_info = si
    return remaining


def _split_multi_waits(nc):
    """Any instruction still carrying >1 ge-waits after relaxation gets its
    excess waits hoisted onto same-engine NoOps inserted right before it
    (a sequence of single-wait instructions is semantically identical to one
    multi-wait instruction on an in-order sequencer)."""
    import bass_rust

    def wkey(w):
        return (w.id, w.wait_value, w.wait_mode)

    plan = {}
    for fn in nc.m.functions:
        for blk in fn.blocks:
            for inst in blk.instructions:
                si = getattr(inst, "sync_info", None)
                if si is None or not si.on_wait:
                    continue
                ow = list(si.on_wait)
                ge = [w for w in ow
                      if w.sync_type == "semaphore" and w.wait_mode == "sem-ge-imm"
                      and w.wait_reg is None
                      and not w.ant_name.startswith("barrier_")]
                if len(ge) <= 1:
                    continue
                hoist = ge[1:]
                hkeys = {wkey(w) for w in hoist}
                nops = []
                for w in hoist:
                    nb = nc.engines[inst.engine].nop(nofuse=True, hint="wait_split")
                    ni = nb.ins
                    ni.sync_info = bass_rust.SyncInfo(on_wait=[w], on_update=[])
                    nops.append(ni)
                plan[inst.name] = nops
                si.on_wait = [w for w in ow if wkey(w) not in hkeys
                              or (w.sync_type, w.wait_mode) != ("semaphore", "sem-ge-imm")]
                inst.sync_info = si
    if not plan:
        return 0
    created = {n.name for nops in plan.values() for n in nops}
    for fn in nc.m.functions:
        for blk in fn.blocks:
            cur = list(blk.instructions)
            new = []
            for i in cur:
                if i.name in created:
                    continue
                if i.name in plan:
                    new.extend(plan[i.name])
                new.append(i)
            blk.instructions = new
    return len(plan)


def get_nc(repeat=1):
    key = ("nc", repeat)
    if key not in _CACHE:
        nc = _build_nc(repeat)
        _relax_waits(nc)
        _split_multi_waits(nc)
        _CACHE[key] = nc
    return _CACHE[key]


def prep_inputs(x, W_qkv, b_qkv):
    """Host-side sharding: returns the 8 per-core input maps."""
    x = np.asarray(x, dtype=np.float32)
    W_qkv = np.asarray(W_qkv, dtype=np.float32)
    b_qkv = np.asarray(b_qkv, dtype=np.float32)
    ones = np.ones((1, S), np.float32)
    in_maps = []
    for c in range(8):
        b, g = divmod(c, 4)
        xTm = np.concatenate([np.ascontiguousarray(x[b].T), ones], axis=0).astype(np.float16)
        heads = list(range(HL * g, HL * g + HL))
        cols = np.concatenate([np.arange(h * Dh, (h + 1) * Dh) for h in heads])
        wqk_m = np.empty((KC, 512), np.float16)
        wqk_m[:D, :256] = W_qkv[:, cols]
        wqk_m[D, :256] = b_qkv[cols]
        wqk_m[:D, 256:] = W_qkv[:, D + cols]
        wqk_m[D, 256:] = b_qkv[D + cols]
        wv_m = np.zeros((KC, VW), np.float16)
        for i, h in enumerate(heads):
            vcols = 2 * D + h * Dh
            wv_m[:D, i * 65:i * 65 + 64] = W_qkv[:, vcols:vcols + Dh]
            wv_m[D, i * 65:i * 65 + 64] = b_qkv[vcols:vcols + Dh]
            wv_m[D, i * 65 + 64] = 1.0  # generates the constant-1 denom column
        in_maps.append({"xT": xTm, "wqk": wqk_m, "wv": wv_m})
    return in_maps


def assemble_output(results):
    out = np.empty((B, S, D), np.float32)
    for c in range(8):
        b, g = divmod(c, 4)
        out[b, :, g * GW:(g + 1) * GW] = results[c]["out"].T
    return out


def kernel(x, W_qkv, b_qkv):
    from concourse.bass_utils import run_bass_kernel_spmd

    nc = get_nc()
    in_maps = prep_inputs(x, W_qkv, b_qkv)
    res = run_bass_kernel_spmd(nc, in_maps, list(range(8)))
    return assemble_output(res.results)
